# revision 2
# baseline (speedup 1.0000x reference)
"""GATv2 stack (3 layers + MLP head) on 8 Trainium2 NeuronCores — v2.

Design vs the v1 baseline:
- Node phase computes xl/xr only for the core's OWN 6272 nodes (one fused
  [128,256] matmul per 128-node tile); the full 50176-row xl table is then
  assembled with a per-layer AllGather (the baseline recomputed the full
  table on every core, 8x redundant work, and needed the full x uploaded
  to every core).
- Edge phase gathers only xl[src] (per-column indirect DMA, the one proven
  gather primitive on this runtime). xr[dst] is never gathered: the
  per-edge one-hot dst matrix (needed anyway for the scatter) is
  transposed on the tensor engine and used to expand the supertile's 128
  xr rows via matmul, accumulated in-place into the gathered xl buffer.
  The xr contribution is subtracted back out exactly in the epilogue:
      out[d] = sum_e alpha_e (xl_e + xr_d) - xr_d.
  This halves SWDGE descriptor-generation work, the dominant serial cost.
- Per-edge math (leaky_relu, att dot, softmax weights) runs as whole
  supertile [128, K, 128] DVE/ACT ops instead of per-128-edge-tile ops.
- Inputs are uploaded once per unique input set and kept device-resident
  (jax.device_put); repeat kernel() calls only execute + download.
"""
import os
import sys

sys.path.insert(0, "/opt/trn_rl_repo")

import hashlib

import numpy as np
import ml_dtypes

import concourse.bass as bass
import concourse.tile as tile
from concourse import bacc, mybir

AF = mybir.ActivationFunctionType
ALU = mybir.AluOpType
F32 = mybir.dt.float32
BF16 = mybir.dt.bfloat16
I32 = mybir.dt.int32
BF_NP = ml_dtypes.bfloat16

P = 128
D = 128
D2 = 2 * D
DOUT = 64
N = 50000
NP_ = 50176            # padded nodes: 8 * 49 * 128
PC = 6272              # nodes per core
NST = 49               # super-tiles (128-dst blocks) per core
NCORE = 8
NLAYER = 3
NEG = 0.2
SLAB = 7 * P

_CACHE = {}


LO = 32768             # int16 gather window size
HIOFF = NP_ - LO       # 17408


def _wrap16(a):
    """[n] int16 slots -> [128, n//16] wrapped (slot i at [i%16, i//16],
    replicated over the 8 groups of 16 partitions)."""
    return np.tile(a.reshape(-1, 16).T, (8, 1))


def _prep_edges(edge_index):
    src = np.asarray(edge_index[0], dtype=np.int64)
    dst = np.asarray(edge_index[1], dtype=np.int64)
    core = dst // PC
    stl = (dst % PC) // P
    half = (src >= LO).astype(np.int64)
    key = (core * NST + stl) * 2 + half
    order = np.argsort(key, kind="stable")
    src_s, dst_s = src[order], dst[order]
    counts = np.bincount(order * 0 + key[order],
                         minlength=NCORE * NST * 2).reshape(NCORE, NST, 2)
    starts = np.zeros(NCORE * NST * 2 + 1, np.int64)
    np.cumsum(counts.reshape(-1), out=starts[1:])

    Kh = np.ceil(counts.max(axis=0) / P).astype(np.int64)   # [NST, 2]
    Kh[:, 0] = np.maximum(Kh[:, 0], 1)
    T = Kh.sum(axis=1)                                       # [NST]
    off = np.zeros(NST + 1, np.int64)
    np.cumsum(T, out=off[1:])
    CT = int(off[-1])

    srcidx = np.zeros((NCORE, CT * P), np.int64)   # unrebased (idma path)
    srcw = np.zeros((NCORE, CT * P), np.int64)     # window-rebased (dg path)
    xrw = np.zeros((NCORE, CT * P), np.int64)      # local dst index
    dstloc = np.full((NCORE, CT * P), -1.0, np.float32)

    for c in range(NCORE):
        for s in range(NST):
            for h in range(2):
                k = (c * NST + s) * 2 + h
                sl = slice(starts[k], starts[k + 1])
                n = int(starts[k + 1] - starts[k])
                base = (off[s] + (Kh[s, 0] if h else 0)) * P
                srcidx[c, base:base + n] = src_s[sl]
                srcw[c, base:base + n] = src_s[sl] - (HIOFF if h else 0)
                xrw[c, base:base + n] = dst_s[sl] % PC
                dstloc[c, base:base + n] = dst_s[sl] % P

    def pack(arr, dt):
        # edge slot i -> [i % P, off + i // P]
        return np.stack([arr[c].reshape(-1, P).T.copy().astype(dt)
                         for c in range(NCORE)])

    return {"T": T, "Kh": Kh, "off": off, "CT": CT,
            "srcidx": pack(srcidx, np.int32),
            "srcw": np.stack([_wrap16(srcw[c].astype(np.int16))
                              for c in range(NCORE)]),
            "xrw": np.stack([_wrap16(xrw[c].astype(np.int16))
                             for c in range(NCORE)]),
            "dstloc": pack(dstloc, np.float32)}


def _build_program(T, Kh, off, CT):
    nswq = int(os.environ.get("GAT_NSWQ", "1"))
    nc = bacc.Bacc("TRN2", target_bir_lowering=False, debug=False,
                   enable_asserts=True, num_devices=NCORE,
                   num_swdge_queues=nswq)

    dram = lambda n, s, d, **kw: nc.dram_tensor(n, s, d, **kw).ap()
    # ---- external inputs ----
    xT0own = dram("xT0own", [P, PC], F32, kind="ExternalInput")
    srcidx_d = dram("srcidx", [P, CT], I32, kind="ExternalInput")
    srcw_d = dram("srcw", [P, CT * 8], mybir.dt.int16, kind="ExternalInput")
    xrw_d = dram("xrw", [P, CT * 8], mybir.dt.int16, kind="ExternalInput")
    dl_d = dram("dl", [P, CT], BF16, kind="ExternalInput")
    wlr0 = dram("wlr0", [P, D2], F32, kind="ExternalInput")
    wlrb = dram("wlrb", [2, P, D2], BF16, kind="ExternalInput")
    blr0 = dram("blr0", [1, D2], F32, kind="ExternalInput")
    blrb = dram("blrb", [2, 1, D2], BF16, kind="ExternalInput")
    att_bc = dram("att_bc", [NLAYER, P, D], BF16, kind="ExternalInput")
    biascol = dram("biascol", [NLAYER, P, 1], F32, kind="ExternalInput")
    w1t = dram("w1t", [P, D], F32, kind="ExternalInput")
    b1row = dram("b1row", [1, D], F32, kind="ExternalInput")
    w2t = dram("w2t", [P, DOUT], F32, kind="ExternalInput")
    b2row = dram("b2row", [1, DOUT], F32, kind="ExternalInput")
    iota_in = dram("iota_in", [P, P], BF16, kind="ExternalInput")
    ident_in = dram("ident_in", [P, P], F32, kind="ExternalInput")
    identb_in = dram("identb_in", [P, P], BF16, kind="ExternalInput")
    onescol_in = dram("onescol_in", [P, 1], BF16, kind="ExternalInput")
    onesrow0 = dram("onesrow0", [1, P], F32, kind="ExternalInput")
    onesrowb = dram("onesrowb", [1, P], BF16, kind="ExternalInput")
    epsone_in = dram("epsone", [1, 1], BF16, kind="ExternalInput")

    # ---- internal DRAM ----
    xlo = [dram(f"xlo{i}", [PC, D], BF16) for i in range(NLAYER)]
    xla = [dram(f"xla{i}", [NP_, D], BF16, addr_space="Shared")
           for i in range(NLAYER)]
    xr = [dram(f"xr{i}", [PC, D], BF16) for i in range(NLAYER)]
    xoTb = [dram(f"xoT{i}b", [P, PC], BF16) for i in range(2)]
    xoT2 = dram("xoT2", [P, PC], F32)
    yT = dram("yT", [DOUT, PC], F32, kind="ExternalOutput")

    n_layers = int(os.environ.get("GAT_LAYERS", str(NLAYER)))
    nst = int(os.environ.get("GAT_NST", str(NST)))
    no_edge = bool(int(os.environ.get("GAT_NO_EDGE", "0")))
    no_node = bool(int(os.environ.get("GAT_NO_NODE", "0")))
    xr_dg = os.environ.get("GAT_XR", "mm") == "dg"
    no_gather = bool(int(os.environ.get("GAT_NO_GATHER", "0")))
    no_cc = bool(int(os.environ.get("GAT_NO_CC", "0")))

    with tile.TileContext(nc) as tc:
        with (
            tc.tile_pool(name="const", bufs=1) as cpool,
            tc.tile_pool(name="wts", bufs=1) as wpool,
            tc.tile_pool(name="idxp", bufs=1) as idxp,
            tc.tile_pool(name="slab", bufs=3) as slabp,
            tc.tile_pool(name="nodeio", bufs=4) as niop,
            tc.tile_pool(name="gath", bufs=2) as gathp,
            tc.tile_pool(name="edge", bufs=2) as edgep,
            tc.tile_pool(name="stt", bufs=3) as sttp,
            tc.tile_pool(name="epi", bufs=3) as epip,
            tc.tile_pool(name="psA", bufs=1, space="PSUM") as psA,
            tc.tile_pool(name="psS", bufs=2, space="PSUM") as psS,
            tc.tile_pool(name="psE", bufs=2, space="PSUM") as psE,
        ):
            # constants
            iota_t = cpool.tile([P, P], BF16)
            nc.sync.dma_start(out=iota_t[:], in_=iota_in[:])
            ident_t = cpool.tile([P, P], F32)
            nc.sync.dma_start(out=ident_t[:], in_=ident_in[:])
            identb_t = cpool.tile([P, P], BF16)
            nc.sync.dma_start(out=identb_t[:], in_=identb_in[:])
            onescol_t = cpool.tile([P, 1], BF16)
            nc.sync.dma_start(out=onescol_t[:], in_=onescol_in[:])
            onesrow0_t = cpool.tile([1, P], F32)
            nc.sync.dma_start(out=onesrow0_t[:], in_=onesrow0[:])
            onesrowb_t = cpool.tile([1, P], BF16)
            nc.sync.dma_start(out=onesrowb_t[:], in_=onesrowb[:])
            epsone_t = cpool.tile([1, 1], BF16)
            nc.sync.dma_start(out=epsone_t[:], in_=epsone_in[:])
            # edge tables: loaded once, reused across layers
            use_dg = os.environ.get("GAT_GATHER", "dg") == "dg"
            if use_dg:
                isw_t = idxp.tile([P, CT * 8], mybir.dt.int16)
                nc.sync.dma_start(out=isw_t[:], in_=srcw_d[:])
                xrw_t = idxp.tile([P, CT * 8], mybir.dt.int16)
                nc.sync.dma_start(out=xrw_t[:], in_=xrw_d[:])
            else:
                is_t = idxp.tile([P, CT], I32)
                nc.sync.dma_start(out=is_t[:], in_=srcidx_d[:])
            dl_t = idxp.tile([P, CT], BF16)
            nc.sync.dma_start(out=dl_t[:], in_=dl_d[:])

            def node_phase(li, src_own_ap, dt_mm, wlr_ap, blr_ap, ones_t):
                """own-node xl|xr rows via one fused [128,256] matmul/tile."""
                wlr_t = wpool.tile([P, D2], dt_mm, tag=f"wlr{li}")
                nc.sync.dma_start(out=wlr_t[:], in_=wlr_ap)
                blr_t = wpool.tile([1, D2], dt_mm, tag=f"blr{li}")
                nc.sync.dma_start(out=blr_t[:], in_=blr_ap)
                for sl in range(7):
                    st = slabp.tile([P, SLAB], dt_mm, tag="xslab")
                    nc.sync.dma_start(
                        out=st[:], in_=src_own_ap[:, sl * SLAB:(sl + 1) * SLAB])
                    for t in range(7):
                        j = sl * 7 + t
                        ps = psA.tile([P, D2], F32, tag="psA")
                        nc.tensor.matmul(out=ps[:], lhsT=st[:, t * P:(t + 1) * P],
                                         rhs=wlr_t[:], start=True, stop=False)
                        nc.tensor.matmul(out=ps[:], lhsT=ones_t[:], rhs=blr_t[:],
                                         start=False, stop=True)
                        ot = niop.tile([P, D2], BF16, tag="xlrout")
                        nc.scalar.activation(ot[:], ps[:], AF.Copy)
                        nc.sync.dma_start(out=xlo[li][j * P:(j + 1) * P, :],
                                          in_=ot[:, 0:D])
                        nc.sync.dma_start(out=xr[li][j * P:(j + 1) * P, :],
                                          in_=ot[:, D:D2])

            def edge_phase(li, xl_ap, xr_ap, out_own_ap, out_dt):
                att_t = wpool.tile([P, D], BF16, tag=f"att{li}")
                nc.sync.dma_start(out=att_t[:], in_=att_bc[li])
                bias_t = wpool.tile([P, 1], F32, tag=f"bias{li}")
                nc.sync.dma_start(out=bias_t[:], in_=biascol[li])

                for s in range(nst):
                    kk = int(T[s])
                    c0 = int(off[s])
                    xrblk = gathp.tile([P, D], BF16, tag="xrblk")
                    nc.sync.dma_start(out=xrblk[:],
                                      in_=xr_ap[s * P:(s + 1) * P, :])
                    # one-hot dst matrix for this supertile (pre-softmax)
                    sel_t = edgep.tile([P, kk, D], BF16, tag="sel")
                    iota3 = iota_t[:].unsqueeze(1).to_broadcast([P, kk, D])
                    dl3 = dl_t[:, c0:c0 + kk].unsqueeze(2).to_broadcast(
                        [P, kk, D])
                    nc.vector.tensor_tensor(out=sel_t[:], in0=iota3, in1=dl3,
                                            op=ALU.is_equal)
                    # gather xl[src] rows, then accumulate xr[dst] in place:
                    # vbuf = xl[src] + sel^T @ xr_block
                    vbuf = gathp.tile([P, kk, D], BF16, tag="vbuf")
                    if use_dg:
                        kl, khh = int(Kh[s, 0]), int(Kh[s, 1])
                        if kl and not no_gather:
                            nc.gpsimd.dma_gather(
                                vbuf[:, :kl, :], xl_ap[0:LO, :],
                                isw_t[:, c0 * 8:(c0 + kl) * 8],
                                kl * P, kl * P, D, single_packet=False,
                                queue_num=0)
                        if khh and not no_gather:
                            nc.gpsimd.dma_gather(
                                vbuf[:, kl:, :], xl_ap[HIOFF:HIOFF + LO, :],
                                isw_t[:, (c0 + kl) * 8:(c0 + kk) * 8],
                                khh * P, khh * P, D, single_packet=False,
                                queue_num=min(1, nswq - 1))
                        if xr_dg:
                            xrb = gathp.tile([P, kk, D], BF16, tag="xrb")
                            nc.gpsimd.dma_gather(
                                xrb[:], xr_ap[:, :],
                                xrw_t[:, c0 * 8:(c0 + kk) * 8],
                                kk * P, kk * P, D, single_packet=False)
                            nc.vector.tensor_add(vbuf[:], vbuf[:], xrb[:])
                        else:
                            for j in range(kk):
                                selT_ps = psS.tile([P, D], BF16, tag="selT")
                                nc.tensor.transpose(out=selT_ps[:],
                                                    in_=sel_t[:, j, :],
                                                    identity=identb_t[:])
                                selT_sb = sttp.tile([P, D], BF16,
                                                    tag="selTsb")
                                nc.scalar.activation(selT_sb[:], selT_ps[:],
                                                     AF.Copy)
                                xre_ps = psS.tile([P, D], F32, tag="xre")
                                nc.tensor.matmul(out=xre_ps[:],
                                                 lhsT=selT_sb[:],
                                                 rhs=xrblk[:], start=True,
                                                 stop=True)
                                nc.vector.tensor_add(vbuf[:, j, :],
                                                     vbuf[:, j, :],
                                                     xre_ps[:])
                    else:
                        for j in range(kk):
                            nc.gpsimd.indirect_dma_start(
                                out=vbuf[:, j, :], out_offset=None,
                                in_=xl_ap[:],
                                in_offset=bass.IndirectOffsetOnAxis(
                                    ap=is_t[:, c0 + j:c0 + j + 1], axis=0))
                        for j in range(kk):
                            selT_ps = psS.tile([P, D], BF16, tag="selT")
                            nc.tensor.transpose(out=selT_ps[:],
                                                in_=sel_t[:, j, :],
                                                identity=identb_t[:])
                            selT_sb = sttp.tile([P, D], BF16, tag="selTsb")
                            nc.scalar.activation(selT_sb[:], selT_ps[:],
                                                 AF.Copy)
                            xre_ps = psS.tile([P, D], F32, tag="xre")
                            nc.tensor.matmul(out=xre_ps[:], lhsT=selT_sb[:],
                                             rhs=xrblk[:], start=True,
                                             stop=True)
                            nc.vector.tensor_add(vbuf[:, j, :], vbuf[:, j, :],
                                                 xre_ps[:])
                    # logits = sum_f att * leaky_relu(v);  leaky = max(0.2v, v)
                    lr_t = edgep.tile([P, kk, D], BF16, tag="lr")
                    nc.vector.scalar_tensor_tensor(
                        out=lr_t[:], in0=vbuf[:], scalar=NEG, in1=vbuf[:],
                        op0=ALU.mult, op1=ALU.max)
                    att3 = att_t[:].unsqueeze(1).to_broadcast([P, kk, D])
                    nc.vector.tensor_tensor(out=lr_t[:], in0=lr_t[:], in1=att3,
                                            op=ALU.mult)
                    logits_t = sttp.tile([P, kk], F32, tag="lg")
                    nc.vector.tensor_reduce(out=logits_t[:], in_=lr_t[:],
                                            axis=mybir.AxisListType.X,
                                            op=ALU.add)
                    ex_t = sttp.tile([P, kk], BF16, tag="ex")
                    nc.scalar.activation(ex_t[:], logits_t[:], AF.Exp)
                    # selx = sel * ex  (in place)
                    ex3 = ex_t[:].unsqueeze(2).to_broadcast([P, kk, D])
                    nc.vector.tensor_tensor(out=sel_t[:], in0=sel_t[:], in1=ex3,
                                            op=ALU.mult)

                    psf_t = psE.tile([P, D], F32, tag="psf")
                    psd_t = psE.tile([P, 1], F32, tag="psd", bufs=1)
                    psf = psf_t[:]
                    psd = psd_t[:]
                    for j in range(kk):
                        nc.tensor.matmul(out=psf, lhsT=sel_t[:, j, :],
                                         rhs=vbuf[:, j, :],
                                         start=(j == 0), stop=(j == kk - 1))
                        nc.tensor.matmul(out=psd, lhsT=sel_t[:, j, :],
                                         rhs=onescol_t[:],
                                         start=(j == 0), stop=False)
                    nc.tensor.matmul(out=psd, lhsT=onesrowb_t[:],
                                     rhs=epsone_t[:], start=False, stop=True)
                    # epilogue: out = psf/denom - xr_block, transpose,
                    # relu(. + bias)
                    rec_t = epip.tile([P, 1], F32, tag="rec")
                    nc.vector.reciprocal(rec_t[:], psd)
                    outn = epip.tile([P, D], F32, tag="outn")
                    nc.vector.scalar_tensor_tensor(
                        out=outn[:], in0=psf, scalar=rec_t[:],
                        in1=xrblk[:], op0=ALU.mult, op1=ALU.subtract)
                    tps = psS.tile([P, D], F32, tag="xre")
                    nc.tensor.transpose(out=tps[:], in_=outn[:],
                                        identity=ident_t[:])
                    outT = epip.tile([P, D], out_dt, tag="outT")
                    nc.scalar.activation(outT[:], tps[:], AF.Relu,
                                         bias=bias_t[:])
                    nc.sync.dma_start(
                        out=out_own_ap[:, s * P:(s + 1) * P], in_=outT[:])

            # ---------------- layers ----------------
            for li in range(n_layers):
                if not no_node:
                    if li == 0:
                        node_phase(0, xT0own, F32, wlr0[:], blr0[:],
                                   onesrow0_t)
                    else:
                        node_phase(li, xoTb[li - 1], BF16, wlrb[li - 1],
                                   blrb[li - 1], onesrowb_t)
                if not no_cc:
                    nc.gpsimd.collective_compute(
                        "AllGather", ALU.bypass,
                        replica_groups=[list(range(NCORE))],
                        ins=[xlo[li][:]], outs=[xla[li][:]])
                if not no_edge:
                    if li < NLAYER - 1:
                        edge_phase(li, xla[li], xr[li], xoTb[li], BF16)
                    else:
                        edge_phase(li, xla[li], xr[li], xoT2, F32)

            # ---------------- MLP head ----------------
            w1t_t = wpool.tile([P, D], F32, tag="w1t")
            nc.sync.dma_start(out=w1t_t[:], in_=w1t[:])
            b1_t = wpool.tile([1, D], F32, tag="b1row")
            nc.sync.dma_start(out=b1_t[:], in_=b1row[:])
            w2t_t = wpool.tile([P, DOUT], F32, tag="w2t")
            nc.sync.dma_start(out=w2t_t[:], in_=w2t[:])
            b2_t = wpool.tile([1, DOUT], F32, tag="b2row")
            nc.sync.dma_start(out=b2_t[:], in_=b2row[:])
            for jj in range(NST):
                x3_t = niop.tile([P, P], F32, tag="x3t")
                nc.sync.dma_start(out=x3_t[:], in_=xoT2[:, jj * P:(jj + 1) * P])
                hps_t = psE.tile([P, D], F32, tag="psf")
                hps = hps_t[:]
                nc.tensor.matmul(out=hps, lhsT=w1t_t[:], rhs=x3_t[:],
                                 start=True, stop=False)
                nc.tensor.matmul(out=hps, lhsT=b1_t[:], rhs=onesrow0_t[:],
                                 start=False, stop=True)
                h_t = niop.tile([P, P], F32, tag="ht")
                nc.scalar.activation(h_t[:], hps, AF.Copy)
                yps_t = psE.tile([P, D], F32, tag="psf")
                yps = yps_t[0:DOUT, :]
                nc.tensor.matmul(out=yps, lhsT=w2t_t[:], rhs=h_t[:],
                                 start=True, stop=False)
                nc.tensor.matmul(out=yps, lhsT=b2_t[:], rhs=onesrow0_t[:],
                                 start=False, stop=True)
                y_t = niop.tile([DOUT, P], F32, tag="yt")
                nc.scalar.activation(y_t[:], yps, AF.Copy)
                nc.sync.dma_start(out=yT[:, jj * P:(jj + 1) * P], in_=y_t[:])

    nc.compile()
    return nc


def _make_in_maps(inputs, ep):
    x = np.asarray(inputs["x"], np.float32)
    Wl = np.asarray(inputs["Wl"], np.float32)
    bl = np.asarray(inputs["bl"], np.float32)
    Wr = np.asarray(inputs["Wr"], np.float32)
    br = np.asarray(inputs["br"], np.float32)
    att = np.asarray(inputs["att"], np.float32)
    bias = np.asarray(inputs["bias"], np.float32)
    W1 = np.asarray(inputs["W1"], np.float32)
    b1 = np.asarray(inputs["b1"], np.float32)
    W2 = np.asarray(inputs["W2"], np.float32)
    b2 = np.asarray(inputs["b2"], np.float32)

    xTp = np.zeros((P, NP_), np.float32)
    xTp[:, :N] = x.T
    wlr = np.concatenate([Wl.transpose(0, 2, 1), Wr.transpose(0, 2, 1)],
                         axis=2)          # [L, D, 2D]
    blr = np.concatenate([bl, br], axis=1)  # [L, 2D]
    common = {
        "wlr0": wlr[0].copy(),
        "wlrb": wlr[1:].astype(BF_NP),
        "blr0": blr[0][None, :].copy(),
        "blrb": blr[1:, None, :].astype(BF_NP),
        "att_bc": np.repeat(att[:, None, :], P, axis=1).astype(BF_NP),
        "biascol": bias[:, :, None].copy(),
        "w1t": W1.T.copy(),
        "b1row": b1[None, :].copy(),
        "w2t": W2.T.copy(),
        "b2row": b2[None, :].copy(),
        "iota_in": np.tile(np.arange(P, dtype=np.float32),
                           (P, 1)).astype(BF_NP),
        "ident_in": np.eye(P, dtype=np.float32),
        "identb_in": np.eye(P, dtype=np.float32).astype(BF_NP),
        "onescol_in": np.ones((P, 1), BF_NP),
        "onesrow0": np.ones((1, P), np.float32),
        "onesrowb": np.ones((1, P), BF_NP),
        "epsone": np.full((1, 1), 1e-30, BF_NP),
    }
    in_maps = []
    for c in range(NCORE):
        m = dict(common)
        m["xT0own"] = xTp[:, c * PC:(c + 1) * PC].copy()
        m["srcidx"] = ep["srcidx"][c]
        m["srcw"] = ep["srcw"][c]
        m["xrw"] = ep["xrw"][c]
        m["dl"] = ep["dstloc"][c].astype(BF_NP)
        in_maps.append(m)
    return in_maps


def _make_runner(nc, in_maps):
    """Build a jitted shard_map callable with device-resident inputs."""
    import jax
    from jax.sharding import Mesh, NamedSharding, PartitionSpec
    from jax.experimental.shard_map import shard_map
    from concourse.bass2jax import (
        _bass_exec_p, install_neuronx_cc_hook, partition_id_tensor,
    )

    install_neuronx_cc_hook()
    n_cores = len(in_maps)
    partition_name = (nc.partition_id_tensor.name
                      if nc.partition_id_tensor else None)
    in_names, out_names, out_avals, zero_outs = [], [], [], []
    for alloc in nc.m.functions[0].allocations:
        if not isinstance(alloc, mybir.MemoryLocationSet):
            continue
        name = alloc.memorylocations[0].name
        if alloc.kind == "ExternalInput":
            if name != partition_name:
                in_names.append(name)
        elif alloc.kind == "ExternalOutput":
            out_names.append(name)
            shape = tuple(alloc.tensor_shape)
            dtype = mybir.dt.np(alloc.dtype)
            out_avals.append(jax.core.ShapedArray(shape, dtype))
            zero_outs.append(np.zeros(shape, dtype))
    all_in_names = list(in_names) + list(out_names)
    if partition_name is not None:
        all_in_names.append(partition_name)

    def _body(*args):
        operands = list(args)
        if partition_name is not None:
            operands.append(partition_id_tensor())
        outs = _bass_exec_p.bind(
            *operands, out_avals=tuple(out_avals),
            in_names=tuple(all_in_names), out_names=tuple(out_names),
            lowering_input_output_aliases=(),
            sim_require_finite=False, sim_require_nnan=False, nc=nc)
        return tuple(outs)

    devices = jax.devices()[:n_cores]
    mesh = Mesh(np.asarray(devices), ("core",))
    n_args = len(in_names) + len(out_names)
    sharded = jax.jit(
        shard_map(_body, mesh=mesh,
                  in_specs=(PartitionSpec("core"),) * n_args,
                  out_specs=(PartitionSpec("core"),) * len(out_names),
                  check_rep=False),
        keep_unused=True)

    sh = NamedSharding(mesh, PartitionSpec("core"))
    dev_in = [
        jax.device_put(
            np.concatenate([np.asarray(in_maps[c][k])
                            for c in range(n_cores)], axis=0), sh)
        for k in in_names
    ]
    dev_zero = [
        jax.device_put(
            np.zeros((n_cores * z.shape[0], *z.shape[1:]), z.dtype), sh)
        for z in zero_outs
    ]
    return {"fn": sharded, "dev_in": dev_in, "dev_zero": dev_zero,
            "out_names": out_names}


def _get_state(inputs):
    h = hashlib.md5()
    for k in sorted(inputs):
        h.update(k.encode())
        h.update(np.ascontiguousarray(inputs[k]).tobytes())
    key = h.hexdigest()
    if key not in _CACHE:
        ep = _prep_edges(inputs["edge_index"])
        nc = _build_program(ep["T"], ep["Kh"], ep["off"], ep["CT"])
        in_maps = _make_in_maps(inputs, ep)
        st = _make_runner(nc, in_maps)
        st["nc"] = nc
        st["ep"] = ep
        _CACHE[key] = st
    return _CACHE[key]


def kernel(**inputs):
    st = _get_state(inputs)
    outs = st["fn"](*st["dev_in"], *st["dev_zero"])
    yT = np.asarray(outs[st["out_names"].index("yT")])  # [NCORE*DOUT, PC]
    y = np.zeros((N, DOUT), np.float32)
    for c in range(NCORE):
        sl = yT[c * DOUT:(c + 1) * DOUT].T  # [PC, DOUT]
        lo = c * PC
        hi = min((c + 1) * PC, N)
        y[lo:hi] = sl[: hi - lo]
    return y


# revision 3
# speedup vs baseline: 1.0914x; 1.0914x over previous
"""GATv2 stack (3 layers + MLP head) on 8 Trainium2 NeuronCores — v2.

Design vs the v1 baseline:
- Node phase computes xl/xr only for the core's OWN 6272 nodes (one fused
  [128,256] matmul per 128-node tile); the full 50176-row xl table is then
  assembled with a per-layer AllGather (the baseline recomputed the full
  table on every core, 8x redundant work, and needed the full x uploaded
  to every core).
- Edge phase gathers only xl[src] (per-column indirect DMA, the one proven
  gather primitive on this runtime). xr[dst] is never gathered: the
  per-edge one-hot dst matrix (needed anyway for the scatter) is
  transposed on the tensor engine and used to expand the supertile's 128
  xr rows via matmul, accumulated in-place into the gathered xl buffer.
  The xr contribution is subtracted back out exactly in the epilogue:
      out[d] = sum_e alpha_e (xl_e + xr_d) - xr_d.
  This halves SWDGE descriptor-generation work, the dominant serial cost.
- Per-edge math (leaky_relu, att dot, softmax weights) runs as whole
  supertile [128, K, 128] DVE/ACT ops instead of per-128-edge-tile ops.
- Inputs are uploaded once per unique input set and kept device-resident
  (jax.device_put); repeat kernel() calls only execute + download.
"""
import os
import sys

sys.path.insert(0, "/opt/trn_rl_repo")

import hashlib

import numpy as np
import ml_dtypes

import concourse.bass as bass
import concourse.tile as tile
from concourse import bacc, mybir

AF = mybir.ActivationFunctionType
ALU = mybir.AluOpType
F32 = mybir.dt.float32
BF16 = mybir.dt.bfloat16
I32 = mybir.dt.int32
BF_NP = ml_dtypes.bfloat16

P = 128
D = 128
D2 = 2 * D
DOUT = 64
N = 50000
NP_ = 50176            # padded nodes: 8 * 49 * 128
PC = 6272              # nodes per core
NST = 49               # super-tiles (128-dst blocks) per core
NCORE = 8
NLAYER = 3
NEG = 0.2
SLAB = 7 * P

_CACHE = {}


LO = 32768             # int16 gather window size
HIOFF = NP_ - LO       # 17408


def _wrap16(a):
    """[n] int16 slots -> [128, n//16] wrapped (slot i at [i%16, i//16],
    replicated over the 8 groups of 16 partitions)."""
    return np.tile(a.reshape(-1, 16).T, (8, 1))


def _prep_edges(edge_index):
    src = np.asarray(edge_index[0], dtype=np.int64)
    dst = np.asarray(edge_index[1], dtype=np.int64)
    core = dst // PC
    stl = (dst % PC) // P
    half = (src >= LO).astype(np.int64)
    key = (core * NST + stl) * 2 + half
    order = np.argsort(key, kind="stable")
    src_s, dst_s = src[order], dst[order]
    counts = np.bincount(order * 0 + key[order],
                         minlength=NCORE * NST * 2).reshape(NCORE, NST, 2)
    starts = np.zeros(NCORE * NST * 2 + 1, np.int64)
    np.cumsum(counts.reshape(-1), out=starts[1:])

    Kh = np.ceil(counts.max(axis=0) / P).astype(np.int64)   # [NST, 2]
    Kh[:, 0] = np.maximum(Kh[:, 0], 1)
    T = Kh.sum(axis=1)                                       # [NST]
    off = np.zeros(NST + 1, np.int64)
    np.cumsum(T, out=off[1:])
    CT = int(off[-1])

    srcidx = np.zeros((NCORE, CT * P), np.int64)   # unrebased (idma path)
    srcw = np.zeros((NCORE, CT * P), np.int64)     # window-rebased (dg path)
    xrw = np.zeros((NCORE, CT * P), np.int64)      # local dst index
    dstloc = np.full((NCORE, CT * P), -1.0, np.float32)

    for c in range(NCORE):
        for s in range(NST):
            for h in range(2):
                k = (c * NST + s) * 2 + h
                sl = slice(starts[k], starts[k + 1])
                n = int(starts[k + 1] - starts[k])
                base = (off[s] + (Kh[s, 0] if h else 0)) * P
                srcidx[c, base:base + n] = src_s[sl]
                srcw[c, base:base + n] = src_s[sl] - (HIOFF if h else 0)
                xrw[c, base:base + n] = dst_s[sl] % PC
                dstloc[c, base:base + n] = dst_s[sl] % P

    def pack(arr, dt):
        # edge slot i -> [i % P, off + i // P]
        return np.stack([arr[c].reshape(-1, P).T.copy().astype(dt)
                         for c in range(NCORE)])

    return {"T": T, "Kh": Kh, "off": off, "CT": CT,
            "dlraw": dstloc.copy(),
            "srcidx": pack(srcidx, np.int32),
            "srcw": np.stack([_wrap16(srcw[c].astype(np.int16))
                              for c in range(NCORE)]),
            "xrw": np.stack([_wrap16(xrw[c].astype(np.int16))
                             for c in range(NCORE)]),
            "dstloc": pack(dstloc, np.float32)}


def _build_program(T, Kh, off, CT):
    nswq = int(os.environ.get("GAT_NSWQ", "1"))
    nc = bacc.Bacc("TRN2", target_bir_lowering=False, debug=False,
                   enable_asserts=True, num_devices=NCORE,
                   num_swdge_queues=nswq)

    dram = lambda n, s, d, **kw: nc.dram_tensor(n, s, d, **kw).ap()
    # ---- external inputs ----
    xT0own = dram("xT0own", [P, PC], F32, kind="ExternalInput")
    srcidx_d = dram("srcidx", [P, CT], I32, kind="ExternalInput")
    srcw_d = dram("srcw", [P, CT * 8], mybir.dt.int16, kind="ExternalInput")
    xrw_d = dram("xrw", [P, CT * 8], mybir.dt.int16, kind="ExternalInput")
    dl_d = dram("dl", [P, CT], BF16, kind="ExternalInput")
    wlr0 = dram("wlr0", [P, D2], F32, kind="ExternalInput")
    wlrb = dram("wlrb", [2, P, D2], BF16, kind="ExternalInput")
    blr0 = dram("blr0", [1, D2], F32, kind="ExternalInput")
    blrb = dram("blrb", [2, 1, D2], BF16, kind="ExternalInput")
    att_bc = dram("att_bc", [NLAYER, P, D], BF16, kind="ExternalInput")
    biascol = dram("biascol", [NLAYER, P, 1], F32, kind="ExternalInput")
    w1t = dram("w1t", [P, D], F32, kind="ExternalInput")
    b1row = dram("b1row", [1, D], F32, kind="ExternalInput")
    w2t = dram("w2t", [P, DOUT], F32, kind="ExternalInput")
    b2row = dram("b2row", [1, DOUT], F32, kind="ExternalInput")
    iota_in = dram("iota_in", [P, P], BF16, kind="ExternalInput")
    iotacol_in = dram("iotacol_in", [P, 1], BF16, kind="ExternalInput")
    dlTb_d = dram("dlTb", [P, CT * P], BF16, kind="ExternalInput")
    ident_in = dram("ident_in", [P, P], F32, kind="ExternalInput")
    identb_in = dram("identb_in", [P, P], BF16, kind="ExternalInput")
    onescol_in = dram("onescol_in", [P, 1], BF16, kind="ExternalInput")
    onesrow0 = dram("onesrow0", [1, P], F32, kind="ExternalInput")
    onesrowb = dram("onesrowb", [1, P], BF16, kind="ExternalInput")
    epsone_in = dram("epsone", [1, 1], BF16, kind="ExternalInput")

    # ---- internal DRAM ----
    xlo = [dram(f"xlo{i}", [PC, D], BF16) for i in range(NLAYER)]
    xla = [dram(f"xla{i}", [NP_, D], BF16, addr_space="Shared")
           for i in range(NLAYER)]
    xr = [dram(f"xr{i}", [PC, D], BF16) for i in range(NLAYER)]
    xoTb = [dram(f"xoT{i}b", [P, PC], BF16) for i in range(2)]
    xoT2 = dram("xoT2", [P, PC], F32)
    yT = dram("yT", [DOUT, PC], F32, kind="ExternalOutput")

    n_layers = int(os.environ.get("GAT_LAYERS", str(NLAYER)))
    nst = int(os.environ.get("GAT_NST", str(NST)))
    no_edge = bool(int(os.environ.get("GAT_NO_EDGE", "0")))
    no_node = bool(int(os.environ.get("GAT_NO_NODE", "0")))
    xr_dg = os.environ.get("GAT_XR", "mm") == "dg"
    no_gather = bool(int(os.environ.get("GAT_NO_GATHER", "0")))
    no_cc = bool(int(os.environ.get("GAT_NO_CC", "0")))

    with tile.TileContext(nc) as tc:
        with (
            tc.tile_pool(name="const", bufs=1) as cpool,
            tc.tile_pool(name="wts", bufs=1) as wpool,
            tc.tile_pool(name="idxp", bufs=1) as idxp,
            tc.tile_pool(name="slab", bufs=3) as slabp,
            tc.tile_pool(name="nodeio", bufs=4) as niop,
            tc.tile_pool(name="gath", bufs=2) as gathp,
            tc.tile_pool(name="edge", bufs=2) as edgep,
            tc.tile_pool(name="stt", bufs=3) as sttp,
            tc.tile_pool(name="epi", bufs=3) as epip,
            tc.tile_pool(name="psA", bufs=1, space="PSUM") as psA,
            tc.tile_pool(name="psS", bufs=4, space="PSUM") as psS,
            tc.tile_pool(name="psE", bufs=2, space="PSUM") as psE,
        ):
            # constants
            iota_t = cpool.tile([P, P], BF16)
            nc.sync.dma_start(out=iota_t[:], in_=iota_in[:])
            iotacol_t = cpool.tile([P, 1], BF16)
            nc.sync.dma_start(out=iotacol_t[:], in_=iotacol_in[:])
            ident_t = cpool.tile([P, P], F32)
            nc.sync.dma_start(out=ident_t[:], in_=ident_in[:])
            identb_t = cpool.tile([P, P], BF16)
            nc.sync.dma_start(out=identb_t[:], in_=identb_in[:])
            onescol_t = cpool.tile([P, 1], BF16)
            nc.sync.dma_start(out=onescol_t[:], in_=onescol_in[:])
            onesrow0_t = cpool.tile([1, P], F32)
            nc.sync.dma_start(out=onesrow0_t[:], in_=onesrow0[:])
            onesrowb_t = cpool.tile([1, P], BF16)
            nc.sync.dma_start(out=onesrowb_t[:], in_=onesrowb[:])
            epsone_t = cpool.tile([1, 1], BF16)
            nc.sync.dma_start(out=epsone_t[:], in_=epsone_in[:])
            # edge tables: loaded once, reused across layers
            use_dg = os.environ.get("GAT_GATHER", "dg") == "dg"
            if use_dg:
                isw_t = idxp.tile([P, CT * 8], mybir.dt.int16)
                nc.sync.dma_start(out=isw_t[:], in_=srcw_d[:])
                if xr_dg:
                    xrw_t = idxp.tile([P, CT * 8], mybir.dt.int16)
                    nc.sync.dma_start(out=xrw_t[:], in_=xrw_d[:])
            else:
                is_t = idxp.tile([P, CT], I32)
                nc.sync.dma_start(out=is_t[:], in_=srcidx_d[:])
            dl_t = idxp.tile([P, CT], BF16)
            nc.sync.dma_start(out=dl_t[:], in_=dl_d[:])

            def node_phase(li, src_own_ap, dt_mm, wlr_ap, blr_ap, ones_t):
                """own-node xl|xr rows via one fused [128,256] matmul/tile."""
                wlr_t = wpool.tile([P, D2], dt_mm, tag=f"wlr{li}")
                nc.sync.dma_start(out=wlr_t[:], in_=wlr_ap)
                blr_t = wpool.tile([1, D2], dt_mm, tag=f"blr{li}")
                nc.sync.dma_start(out=blr_t[:], in_=blr_ap)
                for sl in range(7):
                    st = slabp.tile([P, SLAB], dt_mm, tag="xslab")
                    nc.sync.dma_start(
                        out=st[:], in_=src_own_ap[:, sl * SLAB:(sl + 1) * SLAB])
                    for t in range(7):
                        j = sl * 7 + t
                        ps = psA.tile([P, D2], F32, tag="psA")
                        nc.tensor.matmul(out=ps[:], lhsT=st[:, t * P:(t + 1) * P],
                                         rhs=wlr_t[:], start=True, stop=False)
                        nc.tensor.matmul(out=ps[:], lhsT=ones_t[:], rhs=blr_t[:],
                                         start=False, stop=True)
                        ot = niop.tile([P, D2], BF16, tag="xlrout")
                        nc.scalar.activation(ot[:], ps[:], AF.Copy)
                        nc.sync.dma_start(out=xlo[li][j * P:(j + 1) * P, :],
                                          in_=ot[:, 0:D])
                        nc.sync.dma_start(out=xr[li][j * P:(j + 1) * P, :],
                                          in_=ot[:, D:D2])

            def edge_phase(li, xl_ap, xr_ap, out_own_ap, out_dt):
                att_t = wpool.tile([P, D], BF16, tag=f"att{li}")
                nc.sync.dma_start(out=att_t[:], in_=att_bc[li])
                bias_t = wpool.tile([P, 1], F32, tag=f"bias{li}")
                nc.sync.dma_start(out=bias_t[:], in_=biascol[li])

                for s in range(nst):
                    kk = int(T[s])
                    c0 = int(off[s])
                    xrblk = gathp.tile([P, D], BF16, tag="xrblk")
                    nc.sync.dma_start(out=xrblk[:],
                                      in_=xr_ap[s * P:(s + 1) * P, :])
                    # one-hot dst matrix for this supertile (pre-softmax)
                    sel_t = edgep.tile([P, kk, D], BF16, tag="sel")
                    iota3 = iota_t[:].unsqueeze(1).to_broadcast([P, kk, D])
                    dl3 = dl_t[:, c0:c0 + kk].unsqueeze(2).to_broadcast(
                        [P, kk, D])
                    nc.vector.tensor_tensor(out=sel_t[:], in0=iota3, in1=dl3,
                                            op=ALU.is_equal)
                    # gather xl[src] rows, then accumulate xr[dst] in place:
                    # vbuf = xl[src] + sel^T @ xr_block
                    vbuf = gathp.tile([P, kk, D], BF16, tag="vbuf")
                    if use_dg:
                        kl, khh = int(Kh[s, 0]), int(Kh[s, 1])
                        if kl and not no_gather:
                            nc.gpsimd.dma_gather(
                                vbuf[:, :kl, :], xl_ap[0:LO, :],
                                isw_t[:, c0 * 8:(c0 + kl) * 8],
                                kl * P, kl * P, D, single_packet=False,
                                queue_num=0)
                        if khh and not no_gather:
                            nc.gpsimd.dma_gather(
                                vbuf[:, kl:, :], xl_ap[HIOFF:HIOFF + LO, :],
                                isw_t[:, (c0 + kl) * 8:(c0 + kk) * 8],
                                khh * P, khh * P, D, single_packet=False,
                                queue_num=min(1, nswq - 1))
                        if xr_dg:
                            xrb = gathp.tile([P, kk, D], BF16, tag="xrb")
                            nc.gpsimd.dma_gather(
                                xrb[:], xr_ap[:, :],
                                xrw_t[:, c0 * 8:(c0 + kk) * 8],
                                kk * P, kk * P, D, single_packet=False)
                            nc.vector.tensor_add(vbuf[:], vbuf[:], xrb[:])
                        else:
                            dlTb_t = edgep.tile([P, kk, D], BF16, tag="dlTb")
                            nc.sync.dma_start(
                                out=dlTb_t[:],
                                in_=dlTb_d[:, c0 * P:(c0 + kk) * P].rearrange(
                                    "p (a b) -> p a b", a=kk))
                            selT_sb = edgep.tile([P, kk, D], BF16, tag="selT")
                            ioc3 = iotacol_t[:].unsqueeze(1).to_broadcast(
                                [P, kk, D])
                            nc.vector.tensor_tensor(out=selT_sb[:], in0=ioc3,
                                                    in1=dlTb_t[:],
                                                    op=ALU.is_equal)
                            for j in range(kk):
                                xre_ps = psS.tile([P, D], F32, tag="xre")
                                nc.tensor.matmul(out=xre_ps[:],
                                                 lhsT=selT_sb[:, j, :],
                                                 rhs=xrblk[:], start=True,
                                                 stop=True)
                                nc.vector.tensor_add(vbuf[:, j, :],
                                                     vbuf[:, j, :],
                                                     xre_ps[:])
                    else:
                        for j in range(kk):
                            nc.gpsimd.indirect_dma_start(
                                out=vbuf[:, j, :], out_offset=None,
                                in_=xl_ap[:],
                                in_offset=bass.IndirectOffsetOnAxis(
                                    ap=is_t[:, c0 + j:c0 + j + 1], axis=0))
                        for j in range(kk):
                            selT_ps = psS.tile([P, D], BF16, tag="selT")
                            nc.tensor.transpose(out=selT_ps[:],
                                                in_=sel_t[:, j, :],
                                                identity=identb_t[:])
                            selT_sb = sttp.tile([P, D], BF16, tag="selTsb")
                            nc.scalar.activation(selT_sb[:], selT_ps[:],
                                                 AF.Copy)
                            xre_ps = psS.tile([P, D], F32, tag="xre")
                            nc.tensor.matmul(out=xre_ps[:], lhsT=selT_sb[:],
                                             rhs=xrblk[:], start=True,
                                             stop=True)
                            nc.vector.tensor_add(vbuf[:, j, :], vbuf[:, j, :],
                                                 xre_ps[:])
                    # logits = sum_f att * leaky_relu(v);  leaky = max(0.2v, v)
                    lr_t = edgep.tile([P, kk, D], BF16, tag="lr")
                    nc.vector.scalar_tensor_tensor(
                        out=lr_t[:], in0=vbuf[:], scalar=NEG, in1=vbuf[:],
                        op0=ALU.mult, op1=ALU.max)
                    att3 = att_t[:].unsqueeze(1).to_broadcast([P, kk, D])
                    nc.vector.tensor_tensor(out=lr_t[:], in0=lr_t[:], in1=att3,
                                            op=ALU.mult)
                    logits_t = sttp.tile([P, kk], F32, tag="lg")
                    nc.vector.tensor_reduce(out=logits_t[:], in_=lr_t[:],
                                            axis=mybir.AxisListType.X,
                                            op=ALU.add)
                    ex_t = sttp.tile([P, kk], BF16, tag="ex")
                    nc.scalar.activation(ex_t[:], logits_t[:], AF.Exp)
                    # selx = sel * ex  (in place)
                    ex3 = ex_t[:].unsqueeze(2).to_broadcast([P, kk, D])
                    nc.vector.tensor_tensor(out=sel_t[:], in0=sel_t[:], in1=ex3,
                                            op=ALU.mult)

                    psf_t = psE.tile([P, D], F32, tag="psf")
                    psd_t = psE.tile([P, 1], F32, tag="psd", bufs=1)
                    psf = psf_t[:]
                    psd = psd_t[:]
                    for j in range(kk):
                        nc.tensor.matmul(out=psf, lhsT=sel_t[:, j, :],
                                         rhs=vbuf[:, j, :],
                                         start=(j == 0), stop=(j == kk - 1))
                        nc.tensor.matmul(out=psd, lhsT=sel_t[:, j, :],
                                         rhs=onescol_t[:],
                                         start=(j == 0), stop=False)
                    nc.tensor.matmul(out=psd, lhsT=onesrowb_t[:],
                                     rhs=epsone_t[:], start=False, stop=True)
                    # epilogue: out = psf/denom - xr_block, transpose,
                    # relu(. + bias)
                    rec_t = epip.tile([P, 1], F32, tag="rec")
                    nc.vector.reciprocal(rec_t[:], psd)
                    outn = epip.tile([P, D], F32, tag="outn")
                    nc.vector.scalar_tensor_tensor(
                        out=outn[:], in0=psf, scalar=rec_t[:],
                        in1=xrblk[:], op0=ALU.mult, op1=ALU.subtract)
                    tps = psS.tile([P, D], F32, tag="xre")
                    nc.tensor.transpose(out=tps[:], in_=outn[:],
                                        identity=ident_t[:])
                    outT = epip.tile([P, D], out_dt, tag="outT")
                    nc.scalar.activation(outT[:], tps[:], AF.Relu,
                                         bias=bias_t[:])
                    nc.sync.dma_start(
                        out=out_own_ap[:, s * P:(s + 1) * P], in_=outT[:])

            # ---------------- layers ----------------
            for li in range(n_layers):
                if not no_node:
                    if li == 0:
                        node_phase(0, xT0own, F32, wlr0[:], blr0[:],
                                   onesrow0_t)
                    else:
                        node_phase(li, xoTb[li - 1], BF16, wlrb[li - 1],
                                   blrb[li - 1], onesrowb_t)
                if not no_cc:
                    nc.gpsimd.collective_compute(
                        "AllGather", ALU.bypass,
                        replica_groups=[list(range(NCORE))],
                        ins=[xlo[li][:]], outs=[xla[li][:]])
                if not no_edge:
                    if li < NLAYER - 1:
                        edge_phase(li, xla[li], xr[li], xoTb[li], BF16)
                    else:
                        edge_phase(li, xla[li], xr[li], xoT2, F32)

            # ---------------- MLP head ----------------
            w1t_t = wpool.tile([P, D], F32, tag="w1t")
            nc.sync.dma_start(out=w1t_t[:], in_=w1t[:])
            b1_t = wpool.tile([1, D], F32, tag="b1row")
            nc.sync.dma_start(out=b1_t[:], in_=b1row[:])
            w2t_t = wpool.tile([P, DOUT], F32, tag="w2t")
            nc.sync.dma_start(out=w2t_t[:], in_=w2t[:])
            b2_t = wpool.tile([1, DOUT], F32, tag="b2row")
            nc.sync.dma_start(out=b2_t[:], in_=b2row[:])
            for jj in range(NST):
                x3_t = niop.tile([P, P], F32, tag="x3t")
                nc.sync.dma_start(out=x3_t[:], in_=xoT2[:, jj * P:(jj + 1) * P])
                hps_t = psE.tile([P, D], F32, tag="psf")
                hps = hps_t[:]
                nc.tensor.matmul(out=hps, lhsT=w1t_t[:], rhs=x3_t[:],
                                 start=True, stop=False)
                nc.tensor.matmul(out=hps, lhsT=b1_t[:], rhs=onesrow0_t[:],
                                 start=False, stop=True)
                h_t = niop.tile([P, P], F32, tag="ht")
                nc.scalar.activation(h_t[:], hps, AF.Copy)
                yps_t = psE.tile([P, D], F32, tag="psf")
                yps = yps_t[0:DOUT, :]
                nc.tensor.matmul(out=yps, lhsT=w2t_t[:], rhs=h_t[:],
                                 start=True, stop=False)
                nc.tensor.matmul(out=yps, lhsT=b2_t[:], rhs=onesrow0_t[:],
                                 start=False, stop=True)
                y_t = niop.tile([DOUT, P], F32, tag="yt")
                nc.scalar.activation(y_t[:], yps, AF.Copy)
                nc.sync.dma_start(out=yT[:, jj * P:(jj + 1) * P], in_=y_t[:])

    nc.compile()
    return nc


def _make_in_maps(inputs, ep):
    x = np.asarray(inputs["x"], np.float32)
    Wl = np.asarray(inputs["Wl"], np.float32)
    bl = np.asarray(inputs["bl"], np.float32)
    Wr = np.asarray(inputs["Wr"], np.float32)
    br = np.asarray(inputs["br"], np.float32)
    att = np.asarray(inputs["att"], np.float32)
    bias = np.asarray(inputs["bias"], np.float32)
    W1 = np.asarray(inputs["W1"], np.float32)
    b1 = np.asarray(inputs["b1"], np.float32)
    W2 = np.asarray(inputs["W2"], np.float32)
    b2 = np.asarray(inputs["b2"], np.float32)

    xTp = np.zeros((P, NP_), np.float32)
    xTp[:, :N] = x.T
    wlr = np.concatenate([Wl.transpose(0, 2, 1), Wr.transpose(0, 2, 1)],
                         axis=2)          # [L, D, 2D]
    blr = np.concatenate([bl, br], axis=1)  # [L, 2D]
    common = {
        "wlr0": wlr[0].copy(),
        "wlrb": wlr[1:].astype(BF_NP),
        "blr0": blr[0][None, :].copy(),
        "blrb": blr[1:, None, :].astype(BF_NP),
        "att_bc": np.repeat(att[:, None, :], P, axis=1).astype(BF_NP),
        "biascol": bias[:, :, None].copy(),
        "w1t": W1.T.copy(),
        "b1row": b1[None, :].copy(),
        "w2t": W2.T.copy(),
        "b2row": b2[None, :].copy(),
        "iota_in": np.tile(np.arange(P, dtype=np.float32),
                           (P, 1)).astype(BF_NP),
        "ident_in": np.eye(P, dtype=np.float32),
        "identb_in": np.eye(P, dtype=np.float32).astype(BF_NP),
        "onescol_in": np.ones((P, 1), BF_NP),
        "iotacol_in": np.arange(P, dtype=np.float32)[:, None].astype(BF_NP),
        "onesrow0": np.ones((1, P), np.float32),
        "onesrowb": np.ones((1, P), BF_NP),
        "epsone": np.full((1, 1), 1e-30, BF_NP),
    }
    in_maps = []
    for c in range(NCORE):
        m = dict(common)
        m["xT0own"] = xTp[:, c * PC:(c + 1) * PC].copy()
        m["srcidx"] = ep["srcidx"][c]
        m["srcw"] = ep["srcw"][c]
        m["xrw"] = ep["xrw"][c]
        m["dlTb"] = np.broadcast_to(
            ep["dlraw"][c].astype(BF_NP)[None, :], (P, ep["dlraw"][c].size))
        m["dl"] = ep["dstloc"][c].astype(BF_NP)
        in_maps.append(m)
    return in_maps


def _make_runner(nc, in_maps):
    """Build a jitted shard_map callable with device-resident inputs."""
    import jax
    from jax.sharding import Mesh, NamedSharding, PartitionSpec
    from jax.experimental.shard_map import shard_map
    from concourse.bass2jax import (
        _bass_exec_p, install_neuronx_cc_hook, partition_id_tensor,
    )

    install_neuronx_cc_hook()
    n_cores = len(in_maps)
    partition_name = (nc.partition_id_tensor.name
                      if nc.partition_id_tensor else None)
    in_names, out_names, out_avals, zero_outs = [], [], [], []
    for alloc in nc.m.functions[0].allocations:
        if not isinstance(alloc, mybir.MemoryLocationSet):
            continue
        name = alloc.memorylocations[0].name
        if alloc.kind == "ExternalInput":
            if name != partition_name:
                in_names.append(name)
        elif alloc.kind == "ExternalOutput":
            out_names.append(name)
            shape = tuple(alloc.tensor_shape)
            dtype = mybir.dt.np(alloc.dtype)
            out_avals.append(jax.core.ShapedArray(shape, dtype))
            zero_outs.append(np.zeros(shape, dtype))
    all_in_names = list(in_names) + list(out_names)
    if partition_name is not None:
        all_in_names.append(partition_name)

    def _body(*args):
        operands = list(args)
        if partition_name is not None:
            operands.append(partition_id_tensor())
        outs = _bass_exec_p.bind(
            *operands, out_avals=tuple(out_avals),
            in_names=tuple(all_in_names), out_names=tuple(out_names),
            lowering_input_output_aliases=(),
            sim_require_finite=False, sim_require_nnan=False, nc=nc)
        return tuple(outs)

    devices = jax.devices()[:n_cores]
    mesh = Mesh(np.asarray(devices), ("core",))
    n_args = len(in_names) + len(out_names)
    sharded = jax.jit(
        shard_map(_body, mesh=mesh,
                  in_specs=(PartitionSpec("core"),) * n_args,
                  out_specs=(PartitionSpec("core"),) * len(out_names),
                  check_rep=False),
        keep_unused=True)

    sh = NamedSharding(mesh, PartitionSpec("core"))
    dev_in = [
        jax.device_put(
            np.concatenate([np.asarray(in_maps[c][k])
                            for c in range(n_cores)], axis=0), sh)
        for k in in_names
    ]
    dev_zero = [
        jax.device_put(
            np.zeros((n_cores * z.shape[0], *z.shape[1:]), z.dtype), sh)
        for z in zero_outs
    ]
    return {"fn": sharded, "dev_in": dev_in, "dev_zero": dev_zero,
            "out_names": out_names}


def _get_state(inputs):
    h = hashlib.md5()
    for k in sorted(inputs):
        h.update(k.encode())
        h.update(np.ascontiguousarray(inputs[k]).tobytes())
    key = h.hexdigest()
    if key not in _CACHE:
        ep = _prep_edges(inputs["edge_index"])
        nc = _build_program(ep["T"], ep["Kh"], ep["off"], ep["CT"])
        in_maps = _make_in_maps(inputs, ep)
        st = _make_runner(nc, in_maps)
        st["nc"] = nc
        st["ep"] = ep
        _CACHE[key] = st
    return _CACHE[key]


def kernel(**inputs):
    st = _get_state(inputs)
    outs = st["fn"](*st["dev_in"], *st["dev_zero"])
    yT = np.asarray(outs[st["out_names"].index("yT")])  # [NCORE*DOUT, PC]
    y = np.zeros((N, DOUT), np.float32)
    for c in range(NCORE):
        sl = yT[c * DOUT:(c + 1) * DOUT].T  # [PC, DOUT]
        lo = c * PC
        hi = min((c + 1) * PC, N)
        y[lo:hi] = sl[: hi - lo]
    return y
